# revision 3
# baseline (speedup 1.0000x reference)
"""Trainium2 Bass kernel: 3x3 stride-1 VALID conv (NHWC, HWIO) + bias + ReLU.

Problem shapes:
  x       (32, 112, 112, 64)  f32
  kernels (3, 3, 64, 128)     f32
  biases  (128,)              f32
  out     (32, 110, 110, 128) f32

Strategy:
  * Data-parallel: 4 images per core across 8 NeuronCores (no collectives).
  * Host pre-packs x into a channel/row-parity-major layout
      X[p, rp, b, w]  with p = (h%2)*64 + c,  rp = h//2
    so adjacent image rows sit on opposite halves of the 128 SBUF
    partitions. A 3x3x64 conv then becomes PSUM-accumulated matmuls
    batching all 4 images into N=440 moving columns.
  * Shifted-copy layouts S_e/S_o (built on-device by SBUF->SBUF DMA)
    hold each needed row with its own 1-column-shifted copy on the
    opposite partition half, so 2 of the 3 leftover K=64 taps per
    output row fuse into one K=128 matmul: 5 streams/row (4xK=128 +
    1xK=64) instead of 6 -> 242k PE cycles instead of 290k.
  * fp16 operands: 2-byte LDWEIGHTS fast path, full 1 col/cycle stream
    rate, fp32 PSUM accumulation, conv error ~3e-4.
  * ScalarE fuses bias+ReLU on the PSUM->SBUF evacuation, emitting
    fp16 (halves output DMA traffic; host converts back to f32).
"""

import numpy as np

import concourse.bass as bass
import concourse.mybir as mybir
from concourse import bacc
from concourse.bass_utils import run_bass_kernel_spmd
from concourse.tile import TileContext

N_CORES = 8
B = 4  # images per core
H = W = 112
C = 64
F = 128
KH = KW = 3
HO = WO = 110
NRP = H // 2  # 56 row pairs per image
A = HO // 2  # 55 output row-parity iterations

F32 = mybir.dt.float32
F16 = mybir.dt.float16
MM_DTYPE = F16

X_ELEMS = NRP * B * W  # per-partition input elements (25088)
O_ELEMS = A * 2 * B * WO  # per-partition output elements (48400)

_TRACE = False
LAST_RESULT = None
_NC_CACHE = None


def _build_bass():
    nc = bacc.Bacc("TRN2", target_bir_lowering=False, debug=False)
    x_d = nc.dram_tensor("x", [128, X_ELEMS], MM_DTYPE, kind="ExternalInput")
    # weights (9 stacked [128,128] lhsT tiles) + fp32 bias packed as the
    # last two fp16 columns (bitcast back to f32 on device)
    w_d = nc.dram_tensor("w", [128, 9 * F + 2], MM_DTYPE, kind="ExternalInput")
    o_d = nc.dram_tensor("o", [128, O_ELEMS], F16, kind="ExternalOutput")

    rpw = B * W  # elems per rowpair per partition (448)
    ow = 2 * B * WO  # output elems per a-iteration (880)

    with TileContext(nc) as tc:
        with (
            tc.tile_pool(name="xres", bufs=1) as xpool,
            tc.tile_pool(name="spool", bufs=1) as spool,
            tc.tile_pool(name="wpool", bufs=1) as wpool,
            tc.tile_pool(name="psum", bufs=8, space="PSUM") as ppool,
            tc.tile_pool(name="opool", bufs=4) as opool,
        ):
            # Scalar ring: runs concurrently with chunk 0 on SWDGE.
            wt = wpool.tile([128, 9 * F + 2], MM_DTYPE)
            nc.scalar.dma_start(out=wt[:], in_=w_d[:])
            bt = wt[:, 9 * F : 9 * F + 2].bitcast(F32)

            # Shifted-copy tiles: S_e slot a holds row 2a+2 (low half) and
            # its w+1 shift (high half); S_o slot a holds row 2a+1 likewise.
            se = spool.tile([128, A * rpw], MM_DTYPE, tag="se")
            so = spool.tile([128, A * rpw], MM_DTYPE, tag="so")
            sev = se[:].rearrange("p (a b w) -> p a b w", a=A, b=B, w=W)
            sov = so[:].rearrange("p (a b w) -> p a b w", a=A, b=B, w=W)

            # Fast-start chunk schedule: small chunks first so the first
            # matmul group can begin ASAP, larger chunks once compute is
            # the slower consumer. Input chunks ride SWDGE (gpsimd) whose
            # DMASW sem lanes are disjoint from the DMAHW lanes used by
            # output DMAs — otherwise round-robin lane reuse makes a late
            # input chunk wait on an output DMA (head-of-line blocking).
            chunk_rps = [1, 1, 2, 4] + [8] * 6
            assert sum(chunk_rps) == NRP
            rp2view = []  # rowpair -> (view, local index)
            for ch, nrp_ch in enumerate(chunk_rps):
                cht = xpool.tile([128, nrp_ch * rpw], MM_DTYPE, tag=f"xch{ch}")
                s = len(rp2view) * rpw
                nc.gpsimd.dma_start(out=cht[:], in_=x_d[:, s : s + nrp_ch * rpw])
                v = cht[:].rearrange("p (rp b w) -> p rp b w", rp=nrp_ch, b=B, w=W)
                # S-build copies for the rowpairs in this chunk, on the SP
                # HWDGE queue (disjoint from input SWDGE + output ACT HWDGE).
                # High halves are flat shifted-by-1 copies: contiguous 7KB
                # descriptor lines; the garbage at (slot, b, 111) is never
                # read (matmuls touch w<=110 there).
                lo, hi = s // rpw, s // rpw + nrp_ch  # rp range [lo, hi)
                # S_e slot a <- rp a+1 low half: slots [lo-1, hi-1) ∩ [0, A)
                e0, e1 = max(lo - 1, 0), min(hi - 1, A)
                if e1 > e0:
                    l0, n = e0 + 1 - lo, e1 - e0
                    nc.sync.dma_start(
                        out=sev[0:64, e0:e1, :, :], in_=v[0:64, l0 : l0 + n, :, :]
                    )
                    nc.sync.dma_start(
                        out=se[64:128, e0 * rpw : (e0 + n) * rpw - 1],
                        in_=cht[0:64, l0 * rpw + 1 : (l0 + n) * rpw],
                    )
                # S_o slot a <- rp a high half: slots [lo, hi) ∩ [0, A)
                o0, o1 = lo, min(hi, A)
                if o1 > o0:
                    l0, n = o0 - lo, o1 - o0
                    nc.sync.dma_start(
                        out=sov[0:64, o0:o1, :, :], in_=v[64:128, l0 : l0 + n, :, :]
                    )
                    nc.sync.dma_start(
                        out=so[64:128, o0 * rpw : (o0 + n) * rpw - 1],
                        in_=cht[64:128, l0 * rpw + 1 : (l0 + n) * rpw],
                    )
                for r in range(nrp_ch):
                    rp2view.append((v, r))

            def xs(lo, hi, rp, kw):
                v, r = rp2view[rp]
                return v[lo:hi, r, :, kw : kw + WO]

            wv = wt[:, 0 : 9 * F].rearrange("p (i f) -> p i f", i=9, f=F)

            # PE pre-warm: ~3.4us of dummy matmuls on zeroed SBUF while the
            # first input chunks are still in flight. Releases the PE_HAM
            # clock gate (cold = 1.2GHz) before real work arrives; the
            # scratch PSUM bank is never read.
            warm = wpool.tile([128, 440], MM_DTYPE)
            nc.gpsimd.memset(warm[:], 0.0)
            wps = ppool.tile([128, B * WO], F32, tag="ps")
            for j in range(8):
                nc.tensor.matmul(
                    wps[:], warm[:, 0:128], warm[:], start=(j == 0), stop=(j == 7)
                )

            GRP = 2  # a-iterations per output DMA (bigger transfers, fewer DMAs)
            for a in range(A):
                ji = a % GRP
                if ji == 0:
                    n_in_g = min(GRP, A - a)
                    ot = opool.tile([128, n_in_g * ow], F16, tag="ot")
                for par in (0, 1):
                    ps = ppool.tile([128, B * WO], F32)
                    psv = ps[:].rearrange("p (b w) -> p b w", b=B)
                    if par == 0:
                        # out row 2a: kh=0,1 -> rows 2a,2a+1 (rowpair a, K=128)
                        # with [k0kw;k1kw]; k2 taps: kw=0,1 via S_e pair
                        # [k2k0;k2k1] (K=128), kw=2 single on rp a+1 low (K=64).
                        mms = [(wv[:, kw, :], xs(0, 128, a, kw)) for kw in range(KW)]
                        mms.append((wv[0:64, 8, :], xs(0, 64, a + 1, 2)))
                        mms.append((wv[:, 6, :], sev[:, a, :, 0:WO]))
                    else:
                        # out row 2a+1: kh=1,2 -> rows 2a+2,2a+3 (rowpair a+1,
                        # K=128) with [k1kw;k2kw]; k0 taps: kw=0,1 via S_o pair
                        # [k0k0;k0k1] (K=128), kw=2 single on rp a high (K=64).
                        mms = [(wv[:, 7, :], sov[:, a, :, 0:WO])]
                        mms.append((wv[64:128, 8, :], xs(64, 128, a, 2)))
                        mms += [(wv[:, 3 + kw, :], xs(0, 128, a + 1, kw)) for kw in range(KW)]
                    for j, (lhsT, rhs) in enumerate(mms):
                        nc.tensor.matmul(
                            psv, lhsT, rhs, start=(j == 0), stop=(j == len(mms) - 1)
                        )
                    nc.scalar.activation(
                        out=ot[:, (ji * 2 + par) * B * WO : (ji * 2 + par + 1) * B * WO],
                        in_=ps[:],
                        func=mybir.ActivationFunctionType.Relu,
                        bias=bt,
                    )
                if ji == n_in_g - 1:
                    # Scalar-engine HWDGE queue: keeps output-DMA triggers
                    # (which wait on ACT results) off the input DMA paths.
                    g0 = a - ji
                    nc.scalar.dma_start(
                        out=o_d[:, g0 * ow : (g0 + n_in_g) * ow], in_=ot[:]
                    )
    nc.compile()
    return nc


def _prep_weights(kernels, biases):
    k = np.asarray(kernels, np.float32)  # (3,3,64,128) HWIO
    ws = []
    for kw in range(KW):  # [k0kw;k1kw] pairs (even out-rows, kh=0/1)
        ws.append(np.concatenate([k[0, kw], k[1, kw]], axis=0))
    for kw in range(KW):  # [k1kw;k2kw] pairs (odd out-rows, kh=1/2)
        ws.append(np.concatenate([k[1, kw], k[2, kw]], axis=0))
    ws.append(np.concatenate([k[2, 0], k[2, 1]], axis=0))  # i=6: S_e pair
    ws.append(np.concatenate([k[0, 0], k[0, 1]], axis=0))  # i=7: S_o pair
    ws.append(np.concatenate([k[2, 2], k[0, 2]], axis=0))  # i=8: kw=2 singles
    wdev = np.stack(ws, axis=1).reshape(128, 9 * F).astype(np.float16)
    # fp32 bias bits carried as two fp16 columns (device bitcasts back)
    bdev = np.asarray(biases, np.float32).reshape(128, 1).view(np.float16)
    return np.ascontiguousarray(np.concatenate([wdev, bdev], axis=1))


def kernel(**inputs):
    global _NC_CACHE, LAST_RESULT
    x = np.asarray(inputs["x"], np.float32).astype(np.float16)
    wdev = _prep_weights(inputs["kernels"], inputs["biases"])

    if _NC_CACHE is None:
        _NC_CACHE = _build_bass()
    nc = _NC_CACHE

    in_maps = []
    for i in range(N_CORES):
        xc = x[i * B : (i + 1) * B]  # [4,112,112,64]
        # [b, rp, par, w, c] -> [par, c, rp, b, w]; partition p = par*64 + c
        xp = xc.reshape(B, NRP, 2, W, C).transpose(2, 4, 1, 0, 3)
        in_maps.append(
            {"x": np.ascontiguousarray(xp).reshape(128, X_ELEMS), "w": wdev}
        )

    LAST_RESULT = run_bass_kernel_spmd(
        nc, in_maps, core_ids=list(range(N_CORES)), trace=_TRACE
    )

    outs = []
    for res in LAST_RESULT.results:
        o = res["o"].astype(np.float32).reshape(F, A, 2, B, WO)
        o = o.transpose(3, 1, 2, 4, 0)
        outs.append(o.reshape(B, HO, WO, F))
    return np.ascontiguousarray(np.concatenate(outs, axis=0))


# revision 8
# speedup vs baseline: 1.3820x; 1.3820x over previous
"""Trainium2 Bass kernel: 3x3 stride-1 VALID conv (NHWC, HWIO) + bias + ReLU.

Problem shapes:
  x       (32, 112, 112, 64)  f32
  kernels (3, 3, 64, 128)     f32
  biases  (128,)              f32
  out     (32, 110, 110, 128) f32

Strategy:
  * Data-parallel: 4 images per core across 8 NeuronCores (no collectives).
  * Host pre-packs x into a channel/row-parity-major layout
      X[p, rp, b, w]  with p = (h%2)*64 + c,  rp = h//2
    so every image row sits on a 64-partition half.
  * TRN2 PE streams K<=64 matmuls at 2 cols/cycle (half-height tiles run
    double rate), so a K=64 tap matmul costs the same MACs/cycle as a
    K=128 pair: the conv is done as 9 independent K=64 tap matmuls per
    output row (N=440 moving cols, all 4 images), PSUM-accumulated.
    Uniform (64,128) PE tile config avoids reconfig bubbles.
  * Tap-major ordering over blocks of 4 row-pair iterations (8 psum
    banks) keeps the same stationary weights for 4 consecutive matmuls
    to amortize LDWEIGHTS; weights are duplicated on both partition
    halves so even/odd rows both find their lhsT at the right base.
  * fp16 operands: fast LDWEIGHTS, fp32 PSUM accumulation, err ~3e-4.
  * ScalarE fuses bias+ReLU on PSUM->SBUF evacuation, emitting fp16
    (halves output DMA traffic; host converts back to f32).
"""

import numpy as np

import concourse.bass as bass
import concourse.mybir as mybir
from concourse import bacc
from concourse.bass_utils import run_bass_kernel_spmd
from concourse.tile import TileContext

N_CORES = 8
B = 4  # images per core
H = W = 112
C = 64
F = 128
KH = KW = 3
HO = WO = 110
NRP = H // 2  # 56 row pairs per image
A = HO // 2  # 55 output row-parity iterations
NA = 1  # a-iterations per block (8 psum banks)

F32 = mybir.dt.float32
F16 = mybir.dt.float16
MM_DTYPE = F16

X_ELEMS = NRP * B * W  # per-partition input elements (25088)
O_ELEMS = A * 2 * B * WO  # per-partition output elements (48400)

_TRACE = False
LAST_RESULT = None
_NC_CACHE = None


def _build_bass():
    nc = bacc.Bacc("TRN2", target_bir_lowering=False, debug=False)
    x_d = nc.dram_tensor("x", [128, X_ELEMS], MM_DTYPE, kind="ExternalInput")
    # weights: 9 tap lhsT tiles [64,128] duplicated on both partition
    # halves + fp32 bias packed as two fp16 columns (bitcast on device)
    w_d = nc.dram_tensor("w", [128, 9 * F + 2], MM_DTYPE, kind="ExternalInput")
    o_d = nc.dram_tensor("o", [128, O_ELEMS], F16, kind="ExternalOutput")

    rpw = B * W  # elems per rowpair per partition (448)
    ow = 2 * B * WO  # output elems per a-iteration (880)

    with TileContext(nc) as tc:
        with (
            tc.tile_pool(name="xres", bufs=1) as xpool,
            tc.tile_pool(name="wpool", bufs=1) as wpool,
            tc.tile_pool(name="psum", bufs=8, space="PSUM") as ppool,
            tc.tile_pool(name="opool", bufs=4) as opool,
        ):
            # Scalar ring: runs concurrently with chunk 0 on SWDGE.
            wt = wpool.tile([128, 9 * F + 2], MM_DTYPE)
            nc.scalar.dma_start(out=wt[:], in_=w_d[:])
            bt = wt[:, 9 * F : 9 * F + 2].bitcast(F32)
            wv = wt[:, 0 : 9 * F].rearrange("p (i f) -> p i f", i=9, f=F)

            # Fast-start chunk schedule: small chunks first so the first
            # matmul group can begin ASAP, larger chunks once compute is
            # the slower consumer. Input chunks ride SWDGE (gpsimd) whose
            # DMASW sem lanes are disjoint from the DMAHW lanes used by
            # output DMAs — otherwise round-robin lane reuse makes a late
            # input chunk wait on an output DMA (head-of-line blocking).
            chunk_rps = [1, 1, 2, 4] + [8] * 6
            assert sum(chunk_rps) == NRP
            rp2view = []  # rowpair -> (view, local index)
            for ch, nrp_ch in enumerate(chunk_rps):
                cht = xpool.tile([128, nrp_ch * rpw], MM_DTYPE, tag=f"xch{ch}")
                s = len(rp2view) * rpw
                nc.gpsimd.dma_start(out=cht[:], in_=x_d[:, s : s + nrp_ch * rpw])
                v = cht[:].rearrange("p (rp b w) -> p rp b w", rp=nrp_ch, b=B, w=W)
                for r in range(nrp_ch):
                    rp2view.append((v, r))

            def xs(lo, rp, kw):
                v, r = rp2view[rp]
                return v[lo : lo + 64, r, :, kw : kw + WO]

            # PE pre-warm: ~3us of dummy K=64 matmuls on zeroed SBUF while
            # the first input chunks are still in flight. Releases the
            # PE_HAM clock gate (cold = 1.2GHz) before real work arrives,
            # in the same 64x128 row-tiled mode as the real matmuls (mode
            # switches drain the array). Alternating row tiles target two
            # distinct scratch PSUM banks (concurrent row tiles must never
            # share a bank). Banks are never read.
            warm = wpool.tile([128, 440], MM_DTYPE)
            nc.gpsimd.memset(warm[:], 0.0)
            wp0 = ppool.tile([128, B * WO], F32, tag="ps", name="ps")
            wp1 = ppool.tile([128, B * WO], F32, tag="ps", name="ps")
            for j in range(8):
                nc.tensor.matmul(
                    wp0[:], warm[0:64, 0:128], warm[0:64, :],
                    start=(j == 0), stop=(j == 7),
                )
                nc.tensor.matmul(
                    wp1[:], warm[64:128, 0:128], warm[64:128, :],
                    start=(j == 0), stop=(j == 7),
                )

            # Two concurrent 64-row PE tiles: tile(0,0) serves K=64 taps
            # reading the low X half, tile(64,0) the high half. Per
            # a-iteration the even out-row 2a accumulates in bank pe_, the
            # odd out-row 2a+1 in bank po_; the interleaved order keeps the
            # two in-flight row tiles always on DIFFERENT psum banks
            # (hardware constraint), while same-tile matmuls serialize.
            # Tap (kh,kw) of out-row h reads image row h+kh at partition
            # half (h+kh)%2, rowpair (h+kh)//2.
            GRP = 2  # a-iterations per output DMA
            for a in range(A):
                ji = a % GRP
                if ji == 0:
                    n_in_g = min(GRP, A - a)
                    ot = opool.tile([128, n_in_g * ow], F16, tag="ot")
                pe_ = ppool.tile([128, B * WO], F32, tag="ps", name="ps")
                po_ = ppool.tile([128, B * WO], F32, tag="ps", name="ps")
                pev = pe_[:].rearrange("p (b w) -> p b w", b=B)
                pov = po_[:].rearrange("p (b w) -> p b w", b=B)
                # phase 1: E taps kh=0 (rp a low) + kh=2 (rp a+1 low) on
                # tile(0,0); O taps kh=0 (rp a high) + kh=2 (rp a+1 high)
                # on tile(64,0). phase 2: kh=1 swaps tiles (E high, O low).
                for kh, kw in [(0, 0), (0, 1), (0, 2), (2, 0), (2, 1), (2, 2),
                               (1, 0), (1, 1), (1, 2)]:
                    j = kh * 3 + kw
                    st = kh == 0 and kw == 0
                    sp = kh == 1 and kw == 2
                    elo = 64 * (kh % 2)
                    olo = 64 - elo
                    nc.tensor.matmul(
                        pev, wv[elo : elo + 64, j, :],
                        xs(elo, a + kh // 2, kw), start=st, stop=sp,
                    )
                    nc.tensor.matmul(
                        pov, wv[olo : olo + 64, j, :],
                        xs(olo, a + (kh + 1) // 2, kw), start=st, stop=sp,
                    )
                nc.scalar.activation(
                    out=ot[:, (ji * 2 + 0) * B * WO : (ji * 2 + 1) * B * WO],
                    in_=pe_[:],
                    func=mybir.ActivationFunctionType.Relu,
                    bias=bt,
                )
                nc.scalar.activation(
                    out=ot[:, (ji * 2 + 1) * B * WO : (ji * 2 + 2) * B * WO],
                    in_=po_[:],
                    func=mybir.ActivationFunctionType.Relu,
                    bias=bt,
                )
                if ji == n_in_g - 1:
                    # Scalar-engine HWDGE queue: keeps output-DMA triggers
                    # (which wait on ACT results) off the input DMA paths.
                    g0 = a - ji
                    nc.scalar.dma_start(
                        out=o_d[:, g0 * ow : (g0 + n_in_g) * ow], in_=ot[:]
                    )
    nc.compile()
    return nc


def _prep_weights(kernels, biases):
    k = np.asarray(kernels, np.float32)  # (3,3,64,128) HWIO
    taps = np.stack([k[kh, kw] for kh in range(KH) for kw in range(KW)], axis=1)
    wdev = taps.reshape(64, 9 * F).astype(np.float16)
    wdev = np.concatenate([wdev, wdev], axis=0)  # duplicate on both halves
    # fp32 bias bits carried as two fp16 columns (device bitcasts back)
    bdev = np.asarray(biases, np.float32).reshape(128, 1).view(np.float16)
    return np.ascontiguousarray(np.concatenate([wdev, bdev], axis=1))


def kernel(**inputs):
    global _NC_CACHE, LAST_RESULT
    x = np.asarray(inputs["x"], np.float32).astype(np.float16)
    wdev = _prep_weights(inputs["kernels"], inputs["biases"])

    if _NC_CACHE is None:
        _NC_CACHE = _build_bass()
    nc = _NC_CACHE

    in_maps = []
    for i in range(N_CORES):
        xc = x[i * B : (i + 1) * B]  # [4,112,112,64]
        # [b, rp, par, w, c] -> [par, c, rp, b, w]; partition p = par*64 + c
        xp = xc.reshape(B, NRP, 2, W, C).transpose(2, 4, 1, 0, 3)
        in_maps.append(
            {"x": np.ascontiguousarray(xp).reshape(128, X_ELEMS), "w": wdev}
        )

    LAST_RESULT = run_bass_kernel_spmd(
        nc, in_maps, core_ids=list(range(N_CORES)), trace=_TRACE
    )

    outs = []
    for res in LAST_RESULT.results:
        o = res["o"].astype(np.float32).reshape(F, A, 2, B, WO)
        o = o.transpose(3, 1, 2, 4, 0)
        outs.append(o.reshape(B, HO, WO, F))
    return np.ascontiguousarray(np.concatenate(outs, axis=0))
